# revision 4
# baseline (speedup 1.0000x reference)
"""Transformer block (LN -> MHA -> residual -> LN -> FFN -> residual) on 8
Trainium2 NeuronCores, data-parallel over batch (one batch element per core,
weights replicated, no collectives).

v2: feature-major residual stream end-to-end — no DRAM transpose bounces on
the critical path.

  - x arrives pre-transposed from the host as [D, T] fp32 ("x" input).  LN
    stats (mean/var over the feature dim = partitions) are computed with PE
    ones-matmuls accumulating over the 8 d-tiles; per-token mu/rstd row
    vectors are broadcast across partitions with rank-1 ones matmuls; the
    standardize runs on GpSimd (otherwise idle).
  - QKV / attention as v1 (S pairs run concurrently in the PE array via
    64-row tile_position halves), but exp consumes a [128, 2, 512] PSUM pair
    in ONE Activation instruction (halves ACT instruction count; ACT is the
    attention-phase bottleneck).
  - out-proj emits y1 feature-major (lhsT=wout, rhs=o_fm), bias on ACT
    (Identity+bias), residual on GpSimd; LN2 is feature-major like LN1, so
    FFN1 needs no activation transpose.
  - FFN2 runs token-major (lhsT=hh tile); the y1 residual reaches it
    token-major via a DRAM bounce (bf16 write + transposing read) that fully
    overlaps the ~200us of FFN matmuls.
  - matmul inputs bf16 (weights pre-cast host-side, LN affine folded into the
    following weight matrix), PSUM accumulation fp32.
"""

import sys

sys.path.insert(0, "/opt/trn_rl_repo")

import numpy as np
import ml_dtypes

import concourse.bass as bass
import concourse.tile as tile
from concourse import mybir
from concourse.bass_utils import run_bass_kernel_spmd
import bass_rust

F32 = mybir.dt.float32
BF16 = mybir.dt.bfloat16

B = 8
T = 1024  # tokens per core
D = 1024
H = 16
HD = 64
F = 4096
EPS = 1e-5
P = 128
TT = T // P  # token tiles
DT = D // P  # d tiles
FT = F // P  # ffn hidden tiles
NT = T // 512  # 512-wide token column tiles
SCALE = HD ** -0.5
RD = 1.0 / D


def _bcast_ap(ap, parts):
    """[n] DRAM AP -> [parts, n] with partition stride 0."""
    return bass.AP(tensor=ap.tensor, offset=ap.offset, ap=[[0, parts]] + list(ap.ap))


def split_excess_waits(nc, max_waits=1):
    """walrus codegen rejects multi-sem-wait ctrl instructions; hoist extra
    waits onto preceding NoOps on the same engine."""
    n_split = 0
    for bb in nc.m.functions[0].blocks:
        insts = list(bb.instructions)
        out = []
        changed = False
        for inst in insts:
            si = inst.sync_info
            if si is not None and len(si.on_wait) > max_waits:
                waits = list(si.on_wait)
                extra, keep = waits[:-max_waits], waits[-max_waits:]
                while extra:
                    chunk, extra = extra[:max_waits], extra[max_waits:]
                    nop = mybir.InstNoOp(name=f"I-waitsplit-{n_split}", ins=[], outs=[])
                    n_split += 1
                    nop.engine = inst.engine
                    nop.sync_info = bass_rust.SyncInfo(on_wait=chunk, on_update=[])
                    out.append(nop)
                inst.sync_info = bass_rust.SyncInfo(
                    on_wait=keep, on_update=list(si.on_update)
                )
                changed = True
            out.append(inst)
        if changed:
            bb.instructions.clear()
            for i in out:
                bb.add_instruction(i)
    return n_split


# test hook: CoreSim has no Gelu; test_sim swaps this for Identity and checks
# against a matching numpy reference
GELU_FUNC = mybir.ActivationFunctionType.Gelu
IDENT = mybir.ActivationFunctionType.Identity
EXP = mybir.ActivationFunctionType.Exp
SQRT = mybir.ActivationFunctionType.Sqrt
ADD = mybir.AluOpType.add
SUB = mybir.AluOpType.subtract
MUL = mybir.AluOpType.mult


def build_program():
    nc = bass.Bass("TRN2", target_bir_lowering=False)

    x_d = nc.dram_tensor("x", [D, T], F32, kind="ExternalInput").ap()
    xbf_d = nc.dram_tensor("xbf", [D, T], BF16, kind="ExternalInput").ap()
    wqkv_d = nc.dram_tensor("wqkv", [D, 3 * D], BF16, kind="ExternalInput").ap()
    bqkv_d = nc.dram_tensor("bqkv", [3 * D], F32, kind="ExternalInput").ap()
    vbias_d = nc.dram_tensor("vbias", [D], BF16, kind="ExternalInput").ap()
    wout_d = nc.dram_tensor("wout", [D, D], BF16, kind="ExternalInput").ap()
    bout_d = nc.dram_tensor("bout", [D], F32, kind="ExternalInput").ap()
    w1_d = nc.dram_tensor("w1", [D, F], BF16, kind="ExternalInput").ap()
    b1_d = nc.dram_tensor("b1", [F], F32, kind="ExternalInput").ap()
    w2_d = nc.dram_tensor("w2", [F, D], BF16, kind="ExternalInput").ap()
    b2_d = nc.dram_tensor("b2", [D], BF16, kind="ExternalInput").ap()
    out_d = nc.dram_tensor("out", [T, D], F32, kind="ExternalOutput").ap()

    with tile.TileContext(nc, pool_alloc_mode="stack") as tc:
        _build_kernel(nc, tc, xbf_d, wqkv_d, bqkv_d, vbias_d, wout_d, bout_d,
                      w1_d, b1_d, w2_d, b2_d, out_d)
    return nc


def _build_kernel(nc, tc, x_d, wqkv_d, bqkv_d, vbias_d, wout_d, bout_d,
                  w1_d, b1_d, w2_d, b2_d, out_d):
    import os

    class _StopBuild(Exception):
        pass

    _phases = os.environ.get("KPHASES", "ACDEFGH")
    _open = []

    def open_pool(name, bufs, space="SBUF"):
        cm = tc.tile_pool(name=name, bufs=bufs, space=space)
        _open.append(cm)
        return cm, cm.__enter__()

    def close_pool(h):
        assert _open and _open[-1] is h
        _open.pop()
        h.__exit__(None, None, None)

    def end_phase(ph):
        if ph not in _phases:
            raise _StopBuild()

    for _rep in range(int(os.environ.get("KREPEAT", "1"))):
        try:
            _build_phases(nc, tc, open_pool, close_pool, end_phase,
                          x_d, wqkv_d, bqkv_d, vbias_d, wout_d, bout_d,
                          w1_d, b1_d, w2_d, b2_d, out_d)
        except _StopBuild:
            pass
        while _open:
            _open[-1].__exit__(None, None, None)
            _open.pop()



def _standardize(nc, dst, src, mu_bc, rstd_bc):
    """dst = (src - mu_bc) * rstd_bc over [P, DT, T]; split across DVE (fast)
    and GpSimd to shorten the serial LN barrier."""
    for dt in range(DT):
        eng = nc.vector if dt < 6 else nc.gpsimd
        eng.tensor_tensor(out=dst[:, dt, :], in0=src[:, dt, :],
                          in1=mu_bc[:], op=SUB)
        eng.tensor_tensor(out=dst[:, dt, :], in0=dst[:, dt, :],
                          in1=rstd_bc[:], op=MUL)


def _ln_stats_fm(nc, pers, ps_pool, sq_pool, src, srcsq, ones_col, ones_row,
                 eps1, mu_bc, rstd_bc, tag):
    """Feature-major LN stats: src [P, DT, T] and srcsq [P, T] per-dt supplier
    are bf16; produces mu_bc/rstd_bc [P, T] bf16 SBUF broadcast tiles (value
    varies along T).  srcsq is a callable dt -> AP (so squares can stream)."""
    mu_f = pers.tile([1, T], F32, tag="ln_mu_f", name=f"mu_f_{tag}")
    tmp_f = pers.tile([1, T], F32, tag="ln_tmp_f", name=f"tmp_f_{tag}")
    mu_b = pers.tile([1, T], BF16, tag="ln_mu_b", name=f"mu_b_{tag}")
    rstd_b = pers.tile([1, T], BF16, tag="ln_rstd_b", name=f"rstd_b_{tag}")
    sum_ps = [ps_pool.tile([P, 512], F32, tag="mm", name=f"sum_{tag}_{c}")
              for c in range(NT)]
    sq_ps = [sq_pool.tile([P, 512], F32, tag="o", name=f"sq_{tag}_{c}")
             for c in range(NT)]
    for dt in range(DT):
        ssq = srcsq(dt)
        for c in range(NT):
            sl = slice(c * 512, (c + 1) * 512)
            nc.tensor.matmul(sum_ps[c][0:1, :], lhsT=ones_col[:],
                             rhs=src[:, dt, sl], start=(dt == 0),
                             stop=(dt == DT - 1))
            nc.tensor.matmul(sq_ps[c][0:1, :], lhsT=ones_col[:],
                             rhs=ssq[:, sl], start=(dt == 0),
                             stop=(dt == DT - 1))
    SQUARE = mybir.ActivationFunctionType.Square
    for c in range(NT):
        sl = slice(c * 512, (c + 1) * 512)
        # small [1, 512] ops split ACT/DVE so neither serializes the barrier
        nc.scalar.activation(out=mu_f[:, sl], in_=sum_ps[c][0:1, :],
                             func=IDENT, scale=RD)
        nc.scalar.activation(out=tmp_f[:, sl], in_=sq_ps[c][0:1, :],
                             func=IDENT, scale=RD)
        nc.scalar.activation(out=mu_b[:, sl], in_=mu_f[:, sl], func=IDENT)
        nc.vector.tensor_tensor(out=mu_f[:, sl], in0=mu_f[:, sl],
                                in1=mu_f[:, sl], op=MUL)
        nc.vector.tensor_tensor(out=tmp_f[:, sl], in0=tmp_f[:, sl],
                                in1=mu_f[:, sl], op=SUB)
        nc.scalar.activation(out=tmp_f[:, sl], in_=tmp_f[:, sl], func=SQRT,
                             bias=eps1[:], scale=1.0)
        nc.vector.reciprocal(out=tmp_f[:, sl], in_=tmp_f[:, sl])
        nc.vector.tensor_copy(out=rstd_b[:, sl], in_=tmp_f[:, sl])
    # broadcast across partitions: rank-1 ones matmul, then copy to SBUF bf16
    for c in range(NT):
        sl = slice(c * 512, (c + 1) * 512)
        for i, (stat_b, dst) in enumerate(((mu_b, mu_bc), (rstd_b, rstd_bc))):
            bc_ps = ps_pool.tile([P, 512], F32, tag="mm", name=f"bc_{tag}_{c}")
            nc.tensor.matmul(bc_ps[:], lhsT=ones_row[:], rhs=stat_b[:, sl],
                             start=True, stop=True)
            if i == 0:
                nc.scalar.activation(out=dst[:, sl], in_=bc_ps[:], func=IDENT)
            else:
                nc.vector.tensor_copy(out=dst[:, sl], in_=bc_ps[:])


def _build_phases(nc, tc, open_pool, close_pool, end_phase,
                  x_d, wqkv_d, bqkv_d, vbias_d, wout_d, bout_d,
                  w1_d, b1_d, w2_d, b2_d, out_d):
    Gelu = GELU_FUNC

    dram_h, dram = open_pool("dram", 1, "DRAM")
    pers_h, pers = open_pool("pers", 1)
    ps_h, ps_pool = open_pool("ps", 2, "PSUM")       # tag "mm": 2 banks
    sps_h, s_pool = open_pool("s_ps", 2, "PSUM")     # tag "s": 2x2 banks
    ops_h, o_pool = open_pool("o_ps", 2, "PSUM")     # tag "o": 2 banks

    eps1 = pers.tile([1, 1], F32, tag="eps1")
    nc.vector.memset(eps1, EPS)
    ones_col = pers.tile([P, 1], BF16, tag="ones_col")
    nc.vector.memset(ones_col, 1.0)
    ones_row = pers.tile([1, P], BF16, tag="ones_row")
    nc.vector.memset(ones_row, 1.0)
    ones64 = pers.tile([1, HD], BF16, tag="ones64")
    nc.vector.memset(ones64, 1.0)
    bqkv_sb = pers.tile([P, 24], F32, tag="bqkv_sb")
    nc.sync.dma_start(out=bqkv_sb[:], in_=bqkv_d.rearrange("(ft p) -> p ft", p=P))
    vb_sb = pers.tile([P, D], BF16, tag="vb_sb")
    nc.gpsimd.dma_start(out=vb_sb[:], in_=_bcast_ap(vbias_d, P))
    boutp = pers.tile([P, DT], F32, tag="boutp")
    nc.gpsimd.dma_start(out=boutp[:], in_=bout_d.rearrange("(dt p) -> p dt", p=P))
    b1_sb = pers.tile([P, FT], F32, tag="b1_sb")
    nc.sync.dma_start(out=b1_sb[:], in_=b1_d.rearrange("(ft p) -> p ft", p=P))
    b2b = pers.tile([P, D], BF16, tag="b2b")
    nc.gpsimd.dma_start(out=b2b[:], in_=_bcast_ap(b2_d, P))
    mu_bc = pers.tile([P, T], BF16, tag="mu_bc")
    rstd_bc = pers.tile([P, T], BF16, tag="rstd_bc")
    y1_fm = pers.tile([P, DT, T], BF16, tag="y1_fm")
    h2_fm = pers.tile([P, DT, T], BF16, tag="h2_fm")

    y1fm_dram = dram.tile([D, T], BF16)

    # ---- outer pools (LIFO stack; inner closes first) ----
    px_h, px = open_pool("px", 1)
    xtb = px.tile([P, DT, T], BF16)
    h_fm = px.tile([P, DT, T], BF16)
    po_h, po = open_pool("po", 1)
    o_fm = po.tile([P, DT, T], BF16)
    pqkv_h, pqkv = open_pool("pqkv", 1)
    q_fm = pqkv.tile([P, TT, T], BF16)
    k_fm = pqkv.tile([P, TT, T], BF16)
    v_aug = pqkv.tile([P, TT, H * (HD + 1)], BF16)
    pwqk_h, pwqk = open_pool("pwqk", 1)
    wqk_sb = pwqk.tile([P, DT, 2 * D], BF16)
    pwv_h, pwv = open_pool("pwv", 1)
    wv_sb = pwv.tile([P, DT, D], BF16)

    # ---- Phase A: input DMAs + LN1 (feature-major) ----
    pa_h, pa = open_pool("pa", 2)
    wqkvr = wqkv_d.rearrange("(dt p) f -> p dt f", p=P)
    for dt in range(DT):
        for hv in range(2):
            hs = slice(hv * 512, (hv + 1) * 512)
            nc.sync.dma_start(out=xtb[:, dt, hs],
                              in_=x_d[dt * P:(dt + 1) * P, hs])
    for dc in range(DT):
        nc.sync.dma_start(out=wv_sb[:, dc, :], in_=wqkvr[:, dc, 2 * D:3 * D])
    for dc in range(DT):
        for hv in range(2):
            hs = slice(hv * D, (hv + 1) * D)
            nc.sync.dma_start(out=wqk_sb[:, dc, hs],
                              in_=wqkvr[:, dc, hv * D:(hv + 1) * D])

    SQUARE = mybir.ActivationFunctionType.Square

    def xsq_supplier(dt):
        t = pa.tile([P, T], BF16, tag="xsq", name=f"xsq_{dt}", bufs=2)
        nc.scalar.activation(out=t[:], in_=xtb[:, dt, :], func=SQUARE)
        return t

    _ln_stats_fm(nc, pers, ps_pool, o_pool, xtb, xsq_supplier, ones_col,
                 ones_row, eps1, mu_bc, rstd_bc, "ln1")
    _standardize(nc, h_fm, xtb, mu_bc, rstd_bc)
    close_pool(pa_h)
    end_phase("A")

    # ---- Phase C: V (token-major, ones-augmented) ----
    v_view = v_aug.rearrange("p t (h c) -> p t h c", c=HD + 1)
    nc.vector.memset(v_view[:, :, :, HD:HD + 1], 1.0)
    vb_view = vb_sb.rearrange("p (h c) -> p h c", c=HD)
    for tt in range(TT):
        for vf in range(2):
            ps = ps_pool.tile([P, 512], F32, tag="mm", name="v_ps")
            for dt in range(DT):
                nc.tensor.matmul(
                    ps[:], lhsT=h_fm[:, dt, tt * P:(tt + 1) * P],
                    rhs=wv_sb[:, dt, vf * 512:(vf + 1) * 512],
                    start=(dt == 0), stop=(dt == DT - 1),
                )
            nc.vector.tensor_tensor(
                out=v_view[:, tt, vf * 8:(vf + 1) * 8, 0:HD],
                in0=ps.rearrange("p (h c) -> p h c", c=HD),
                in1=vb_view[:, vf * 8:(vf + 1) * 8, :],
                op=ADD,
            )
    close_pool(pwv_h)
    end_phase("C")

    # ---- Phase D: attention, fused with Q/K production ----
    attn_h, attn_p = open_pool("attn", 2)
    for hp in range(8):
        for ft in (hp, 8 + hp):
            dst = q_fm if ft < 8 else k_fm
            for nt in range(NT):
                ps = ps_pool.tile([P, 512], F32, tag="mm", name="qk_ps")
                for dt in range(DT):
                    nc.tensor.matmul(
                        ps[:], lhsT=wqk_sb[:, dt, ft * P:(ft + 1) * P],
                        rhs=h_fm[:, dt, nt * 512:(nt + 1) * 512],
                        start=(dt == 0), stop=(dt == DT - 1),
                    )
                nc.vector.tensor_scalar_add(
                    out=dst[:, hp, nt * 512:(nt + 1) * 512], in0=ps[:],
                    scalar1=bqkv_sb[:, ft:ft + 1],
                )
        for nt in range(NT):
            o_ps = [
                o_pool.tile([P, 512], F32, tag="o", name=f"o_ps_{hp}_{nt}_{h}")
                for h in range(2)
            ]
            def issue_s(mt):
                s_pair = s_pool.tile([P, 2, 512], F32, tag="s", name="s_pair")
                for half in range(2):
                    kq = half * HD
                    nc.tensor.matmul(
                        s_pair[:, half, :],
                        lhsT=k_fm[kq:kq + HD, hp, mt * P:(mt + 1) * P],
                        rhs=q_fm[kq:kq + HD, hp, nt * 512:(nt + 1) * 512],
                        start=True, stop=True,
                    )
                return s_pair

            # software pipeline: S(mt+1) issues before AV(mt) so the PE never
            # waits on exp(mt) with nothing to do
            s_pair = issue_s(0)
            for mt in range(TT):
                pt_pair = attn_p.tile([P, 2, 512], BF16, tag="pt", name="pt",
                                      bufs=4)
                nc.scalar.activation(out=pt_pair[:], in_=s_pair[:], func=EXP,
                                     scale=SCALE)
                if mt + 1 < TT:
                    s_pair = issue_s(mt + 1)
                for half in range(2):
                    head = 2 * hp + half
                    nc.tensor.matmul(
                        o_ps[half][0:HD + 1, :],
                        lhsT=v_aug[:, mt, head * (HD + 1):(head + 1) * (HD + 1)],
                        rhs=pt_pair[:, half, :],
                        start=(mt == 0), stop=(mt == TT - 1),
                    )
            for half in range(2):
                rden = attn_p.tile([1, 512], BF16, tag="rden", name="rden")
                with nc.allow_low_precision(reason="bf16 softmax denom ok at 2e-2 tol"):
                    nc.vector.reciprocal(out=rden[:], in_=o_ps[half][HD:HD + 1, :])
                # partition-broadcast via PE rank-1 matmul (engines/DMA cannot
                # broadcast across partitions from on-chip memory)
                bc_ps = ps_pool.tile([P, 512], F32, tag="mm", name="bc_att")
                nc.tensor.matmul(bc_ps[0:HD, :], lhsT=ones64[:], rhs=rden[:],
                                 start=True, stop=True)
                rdenb = attn_p.tile([HD, 512], F32, tag="rdenb", name="rdenb")
                nc.vector.tensor_copy(out=rdenb[:], in_=bc_ps[0:HD, :])
                if half == 0:
                    nc.vector.tensor_tensor(
                        out=o_fm[0:HD, hp, nt * 512:(nt + 1) * 512],
                        in0=o_ps[half][0:HD, :], in1=rdenb[:], op=MUL,
                    )
                else:
                    # compute engines cannot shift partition base; stage at
                    # base 0 then DMA (full crossbar) into partitions 64-127
                    stage = attn_p.tile([HD, 512], BF16, tag="stage",
                                        name="stage")
                    nc.vector.tensor_tensor(
                        out=stage[:], in0=o_ps[half][0:HD, :], in1=rdenb[:],
                        op=MUL,
                    )
                    nc.sync.dma_start(
                        out=o_fm[HD:P, hp, nt * 512:(nt + 1) * 512],
                        in_=stage[:],
                    )
    close_pool(attn_h)
    close_pool(pwqk_h)
    close_pool(pqkv_h)
    end_phase("D")

    # ---- Phase E: out-proj (feature-major) + residual + LN2 stats ----
    y1sq_tiles = []
    pwout_h, pwout = open_pool("pwout", 1)
    woutr = wout_d.rearrange("(dt p) d -> p dt d", p=P)
    wout_blks = []
    for do in range(DT):
        blk = pwout.tile([P, DT, P], BF16, tag="wout_blk", name=f"wout_{do}",
                         bufs=DT)
        nc.sync.dma_start(out=blk[:], in_=woutr[:, :, do * P:(do + 1) * P])
        wout_blks.append(blk)
    pe_h, pe = open_pool("pe", 2)
    for do in range(DT):
        for c in range(NT):
            sl = slice(c * 512, (c + 1) * 512)
            ps = ps_pool.tile([P, 512], F32, tag="mm", name="op_ps")
            for dt in range(DT):
                nc.tensor.matmul(
                    ps[:], lhsT=wout_blks[do][:, dt, :],
                    rhs=o_fm[:, dt, sl],
                    start=(dt == 0), stop=(dt == DT - 1),
                )
            y1a = pe.tile([P, 512], BF16, tag="y1a", name="y1a")
            nc.scalar.activation(out=y1a[:], in_=ps[:], func=IDENT,
                                 bias=boutp[:, do:do + 1], scale=1.0)
            nc.vector.tensor_tensor(out=y1_fm[:, do, sl], in0=y1a[:],
                                    in1=xtb[:, do, sl], op=ADD)
        # bounce y1 to DRAM for the token-major FFN2 residual read
        nc.sync.dma_start(out=y1fm_dram[do * P:(do + 1) * P, :],
                          in_=y1_fm[:, do, :])
        t = pe.tile([P, T], BF16, tag="y1sq", name=f"y1sq_{do}", bufs=DT)
        nc.scalar.activation(out=t[:], in_=y1_fm[:, do, :],
                             func=mybir.ActivationFunctionType.Square)
        y1sq_tiles.append(t)

    def y1sq_supplier(dt):
        return y1sq_tiles[dt]

    _ln_stats_fm(nc, pers, ps_pool, o_pool, y1_fm, y1sq_supplier, ones_col,
                 ones_row, eps1, mu_bc, rstd_bc, "ln2")
    close_pool(pe_h)
    close_pool(pwout_h)
    close_pool(po_h)
    close_pool(px_h)
    end_phase("E")

    # ---- Phase F: LN2 standardize into h2 ----
    _standardize(nc, h2_fm, y1_fm, mu_bc, rstd_bc)
    end_phase("F")

    # ---- Phase G: FFN1 (gelu fused on ACT) ----
    phh_h, phh = open_pool("phh", 1)
    hh_fm = phh.tile([P, FT, T], BF16)
    # ph (FFN2 buffers) opens early so w2 prefetches during FFN1 on the ACT
    # DMA queue
    ph_h, ph_p = open_pool("ph", 2)
    w2r = w2_d.rearrange("(ft p) d -> p ft d", p=P)
    w2_blks = []
    for ot in range(NT):
        sl = slice(ot * 512, (ot + 1) * 512)
        w2_blk = ph_p.tile([P, FT, 512], BF16, tag="w2_blk", bufs=2,
                           name=f"w2_blk_{ot}")
        w2_blks.append(w2_blk)

    def _w2_load(ot):
        sl = slice(ot * 512, (ot + 1) * 512)
        for fc in range(0, FT, 8):
            nc.scalar.dma_start(out=w2_blks[ot][:, fc:fc + 8, :],
                                in_=w2r[:, fc:fc + 8, sl])

    _w2_load(0)
    g_h, g_p = open_pool("g", 2)
    w1r = w1_d.rearrange("(dt p) f -> p dt f", p=P)
    for ft in range(FT):
        if ft == FT // 2:
            _w2_load(1)
        w1_blk = g_p.tile([P, DT, P], BF16, tag="w1_blk", name="w1_blk")
        nc.sync.dma_start(out=w1_blk[:], in_=w1r[:, :, ft * P:(ft + 1) * P])
        pair = s_pool.tile([P, 2, 512], F32, tag="s", name="g_pair")
        for c in range(NT):
            for dt in range(DT):
                nc.tensor.matmul(
                    pair[:, c, :], lhsT=w1_blk[:, dt, :],
                    rhs=h2_fm[:, dt, c * 512:(c + 1) * 512],
                    start=(dt == 0), stop=(dt == DT - 1),
                )
        nc.scalar.activation(
            out=hh_fm[:, ft, :], in_=pair[:], func=Gelu,
            bias=b1_sb[:, ft:ft + 1], scale=1.0,
        )
    close_pool(g_h)
    end_phase("G")

    # ---- Phase H: FFN2 (token-major, tt-outer; y1t streams) ----
    for tt in range(TT):
        y1t = ph_p.tile([P, T], BF16, tag="y1t", name=f"y1t_{tt}", bufs=3)
        nc.sync.dma_start(
            out=y1t[:], in_=y1fm_dram[:, tt * P:(tt + 1) * P],
            transpose=True,
        )
        for ot in range(NT):
            sl = slice(ot * 512, (ot + 1) * 512)
            w2_blk = w2_blks[ot]
            ps = ps_pool.tile([P, 512], F32, tag="mm", name="f2_ps")
            for ft in range(FT):
                nc.tensor.matmul(
                    ps[:], lhsT=hh_fm[:, ft, tt * P:(tt + 1) * P],
                    rhs=w2_blk[:, ft, :],
                    start=(ft == 0), stop=(ft == FT - 1),
                )
            ot_t = ph_p.tile([P, 512], F32, tag="ot_t", name="ot_t")
            nc.vector.tensor_tensor(out=ot_t[:], in0=ps[:], in1=b2b[:, sl],
                                    op=ADD)
            nc.gpsimd.tensor_tensor(out=ot_t[:], in0=ot_t[:],
                                    in1=y1t[:, sl], op=ADD)
            nc.sync.dma_start(out=out_d[tt * P:(tt + 1) * P, sl], in_=ot_t[:])
    close_pool(ph_h)
    close_pool(phh_h)

    close_pool(ops_h)
    close_pool(sps_h)
    close_pool(ps_h)
    close_pool(pers_h)
    close_pool(dram_h)


_NC_CACHE = None


def get_program():
    global _NC_CACHE
    if _NC_CACHE is None:
        _NC_CACHE = build_program()
    return _NC_CACHE


def prepare_in_maps(inputs):
    """Host-side prep: transpose x to feature-major, fold LN affine params
    into the following matmul, cast weights to bf16, build per-core input
    dicts (core b gets batch element b)."""
    f32 = np.float32
    x = np.asarray(inputs["x"], f32)
    qkv_w = np.asarray(inputs["qkv_w"], f32)
    qkv_b = np.asarray(inputs["qkv_b"], f32)
    out_w = np.asarray(inputs["out_w"], f32)
    out_b = np.asarray(inputs["out_b"], f32)
    ffn_w1 = np.asarray(inputs["ffn_w1"], f32)
    ffn_b1 = np.asarray(inputs["ffn_b1"], f32)
    ffn_w2 = np.asarray(inputs["ffn_w2"], f32)
    ffn_b2 = np.asarray(inputs["ffn_b2"], f32)
    ln1_g = np.asarray(inputs["ln1_g"], f32)
    ln1_b = np.asarray(inputs["ln1_b"], f32)
    ln2_g = np.asarray(inputs["ln2_g"], f32)
    ln2_b = np.asarray(inputs["ln2_b"], f32)

    bf = ml_dtypes.bfloat16
    wqkv = np.ascontiguousarray(ln1_g[:, None] * qkv_w).astype(bf)
    bqkv = (qkv_b + ln1_b @ qkv_w).astype(f32)
    w1 = np.ascontiguousarray(ln2_g[:, None] * ffn_w1).astype(bf)
    b1 = (ffn_b1 + ln2_b @ ffn_w1).astype(f32)
    shared = {
        "wqkv": wqkv, "bqkv": bqkv, "vbias": bqkv[2 * D:].astype(bf),
        "wout": out_w.astype(bf), "bout": out_b,
        "w1": w1, "b1": b1,
        "w2": ffn_w2.astype(bf), "b2": ffn_b2.astype(bf),
    }
    xts = [np.ascontiguousarray(x[b].T) for b in range(B)]
    return [{"x": xts[b], "xbf": xts[b].astype(bf), **shared}
            for b in range(B)]


def kernel(**inputs):
    nc = get_program()
    if not getattr(nc, "_waits_split", False):
        # needed for walrus codegen only; CoreSim runs on the unsplit program
        split_excess_waits(nc)
        nc._waits_split = True
    in_maps = prepare_in_maps(inputs)
    res = run_bass_kernel_spmd(nc, in_maps, list(range(B)))
    return np.stack([res.results[b]["out"] for b in range(B)]).astype(np.float32)


if __name__ == "__main__":
    import reference  # only when run manually in the dev dir

    inputs = reference.setup_inputs()
    expected = np.asarray(reference.reference(**inputs))
    actual = kernel(**{k: np.asarray(v) for k, v in inputs.items()})
    err = np.linalg.norm(actual - expected) / np.linalg.norm(expected)
    print("Relative error:", err)
